# revision 1
# baseline (speedup 1.0000x reference)
"""Trainium2 Bass kernel for nn_BayerFeatureExtractor.

Input:  bayer [4, 1, 768, 768] f32.  Output: [4, 30, 768, 768] f32.

Sharding: pure data-parallel over 8 cores: core i handles batch b = i//2,
row-half h = i%2 (output rows [h*384, (h+1)*384)).

Per-core kernel: every convolution is expressed as a small set of banded
matmuls on the TensorEngine (contraction over image rows = SBUF partitions,
float32r at N>=256 streams 1 column/cycle), with Bayer-phase weights folded
into the bands / column-parity interleaves.  Horizontal finishing taps and
elementwise chains run on the Vector/Scalar engines.  Each core processes
4 row-tiles (96 output rows) x 2 col-blocks (384 output cols + halo).

Geometry (validated bit-exact vs reference by a numpy prototype):
  BT = bayer_pad[r0:r0+104, c0:c0+392]; BT partition k <-> out row r0+k-4
  ext maps: M=100, ext row x <-> out row r0-2+x (reads BT partitions x+2+dy)
  stage-2: M=96, out row y reads ext partitions y+2+dy
  stage-1 pure-V psum N=392 (cols c0-4..); direct/ext N=388 (cols c0-2..);
  stage-2 psum N=384 (central).
"""
import math
import os
import sys
from contextlib import ExitStack

import numpy as np

for _p in ('/opt/trn_rl_repo', '/root/.axon_site/_ro/trn_rl_repo'):
    if os.path.isdir(_p) and _p not in sys.path:
        sys.path.insert(0, _p)

import concourse.bass as bass
import concourse.bacc as bacc
import concourse.mybir as mybir
import concourse.tile as tile
from concourse.bass_utils import run_bass_kernel_spmd

F32 = mybir.dt.float32
F32R = mybir.dt.float32r
BF16 = mybir.dt.bfloat16
AL = mybir.AluOpType
AF = mybir.ActivationFunctionType

EPS = 1e-6
K1, M1 = 104, 100
K2, M2 = 100, 96

# ----------------------------------------------------------------------------
# constants (identical math to reference._build_kernels)
# ----------------------------------------------------------------------------


def _norm(k):
    k = k - k.mean()
    return (k / max(float(np.abs(k).sum()), 1e-6)).astype(np.float32)


def _gabor(theta, size=5, sigma=1.1, wavelength=3.0, gamma=0.65):
    r = size // 2
    c = np.arange(-r, r + 1, dtype=np.float32)
    yy, xx = np.meshgrid(c, c, indexing='ij')
    xt = xx * math.cos(theta) + yy * math.sin(theta)
    yt = -xx * math.sin(theta) + yy * math.cos(theta)
    env = np.exp(-(xt ** 2 + (gamma * yt) ** 2) / (2.0 * sigma * sigma))
    return _norm(env * np.cos(2.0 * math.pi * xt / wavelength))


def _dct(size=5, u=2, v=2):
    c = np.arange(size, dtype=np.float32)
    return _norm(np.outer(np.cos(math.pi * (c + 0.5) * v / size),
                          np.cos(math.pi * (c + 0.5) * u / size)))


def build_kernels():
    f32 = np.float32
    k3 = np.stack([
        _norm(np.array([[-1, 0, 1], [-2, 0, 2], [-1, 0, 1]], f32)),
        _norm(np.array([[-1, -2, -1], [0, 0, 0], [1, 2, 1]], f32)),
        _norm(np.array([[-2, -1, 0], [-1, 0, 1], [0, 1, 2]], f32)),
        _norm(np.array([[0, 1, 2], [-1, 0, 1], [-2, -1, 0]], f32)),
        np.array([[0, 1, 0], [1, -4, 1], [0, 1, 0]], f32),
        np.array([[0, 0, 0], [1, -2, 1], [0, 0, 0]], f32),
        np.array([[0, 1, 0], [0, -2, 0], [0, 1, 0]], f32),
        np.array([[1, 0, -1], [0, 0, 0], [-1, 0, 1]], f32) / 4.0,
        np.array([[0, .25, 0], [.25, 0, .25], [0, .25, 0]], f32),
        _norm(np.array([[1, -2, 1], [-2, 4, -2], [1, -2, 1]], f32)),
    ])
    ii, jj = np.indices((5, 5))
    s = np.sin(2.0 * math.pi * np.arange(5, dtype=f32) / 5.0)
    c = np.cos(2.0 * math.pi * np.arange(5, dtype=f32) / 5.0)
    k5 = np.stack([
        _norm(((-1.0) ** (ii + jj)).astype(f32)),   # cb
        _norm(((-1.0) ** jj).astype(f32)),          # sh
        _norm(((-1.0) ** ii).astype(f32)),          # sv
        _norm(np.tile(s, (5, 1))),                  # sinx
        _norm(np.tile(s.reshape(5, 1), (1, 5))),    # siny
        _norm(np.tile(c, (5, 1))),                  # phx
        _norm(np.tile(c.reshape(5, 1), (1, 5))),    # phy
        _gabor(math.pi / 4.0),                      # g45
        _gabor(3.0 * math.pi / 4.0),                # g135
        _dct(),                                     # dct
    ])
    ha = np.array([-0.25, 0.5, 0.5, 0.5, -0.25], f32)
    return k3, k5, ha


def banded1(col5, off=2, M=M1):
    B = np.zeros((K1, M), np.float32)
    for x in range(M):
        for dy in range(-2, 3):
            k = x + off + dy
            if 0 <= k < K1:
                B[k, x] = col5[dy + 2]
    return B


def banded2(col5, scale=1.0):
    B = np.zeros((K2, M2), np.float32)
    for y in range(M2):
        for dy in range(-2, 3):
            k = y + 2 + dy
            if 0 <= k < K2:
                B[k, y] = col5[dy + 2] * scale
    return B


def pad5(col3):
    z = np.zeros(5, np.float32)
    z[1:4] = np.asarray(col3, np.float32)
    return z


def build_stage1_lhs():
    """Stage-1 lhsT matrices.  Two kinds:
    - ext-V (off=2, M=100): V3a, V3b, Vodd, Veven  (psum N=392)
    - central direct (off=4, M=96): everything consumed only centrally
      (psum N=384, rhs col offset 4+dx)
    All matrices packed at column stride M1=100 (central ones zero-padded).
    """
    k3, k5, ha = build_kernels()
    t5 = np.array([1, 2, 3, 2, 1], np.float32) / 9.0
    mats, idx = [], {}

    def add(name, mlist, dxs=None, M=96):
        idx[name] = (len(mats), len(mlist))
        idx[name + '_M'] = M
        if dxs is not None:
            idx[name + '_dx'] = dxs
        mats.extend(mlist)

    add('V3a', [banded1(pad5([1, 2, 1]))], M=100)
    add('V3b', [banded1(pad5([-1, 0, 1]))], M=100)
    Bo = banded1(t5)
    Be = Bo.copy()
    kk = np.arange(K1) % 2
    Bo = Bo * kk[:, None]
    Be = Be * (1 - kk)[:, None]
    add('Vodd', [Bo.astype(np.float32)], M=100)
    add('Veven', [Be.astype(np.float32)], M=100)

    def direct(name, ker):
        kh, kw = ker.shape
        r = kw // 2
        ms, dxs = [], []
        for dx in range(-r, r + 1):
            col = ker[:, dx + r]
            if not np.any(col != 0):
                continue
            c5 = pad5(col) if kh == 3 else col.astype(np.float32)
            ms.append(banded1(c5, off=4, M=96))
            dxs.append(dx)
        add(name, ms, dxs, M=96)

    direct('gdm', k3[2])
    direct('gda', k3[3])
    direct('sumd', k3[0] + k3[1] + k3[2] + k3[3])
    direct('lap', k3[4])
    direct('hd', k3[5] - k3[6])
    direct('hxy', k3[7])
    direct('gcross', k3[8])
    direct('hf', k3[9])
    direct('gxC', k3[0])
    direct('gyC', k3[1])
    for nm, kk5 in zip(['cb', 'sh', 'sv', 'sinx', 'siny', 'phx', 'phy',
                        'g45', 'g135', 'dct'], k5):
        direct(nm, kk5)
    IshC = banded1(pad5([0, 1, 0]), off=4, M=96)
    add('gh', [IshC * ha[dx + 2] for dx in range(-2, 3)],
        [-2, -1, 0, 1, 2], M=96)
    add('gv', [banded1(ha, off=4, M=96)], M=96)
    add('bayC', [IshC], M=96)

    packed = np.zeros((len(mats), K1, M1), np.float32)
    for i, m in enumerate(mats):
        packed[i, :, :m.shape[1]] = m
    return packed, idx


def build_stage2_lhs():
    t5 = np.array([1, 2, 3, 2, 1], np.float32) / 9.0
    ones5 = np.ones(5, np.float32) / 5.0
    I = np.eye(K2, dtype=np.float32)
    a_mats = np.stack([I * t5[dx + 2] for dx in range(-2, 3)])  # [5,100,100]
    j_mats = np.stack([banded2(t5, t5[dx + 2]) for dx in range(-2, 3)])
    box_mat = banded2(ones5, 1.0 / 5.0)
    return a_mats, j_mats, box_mat


PAT_NAMES = ['IVR', 'IVB', 'IVG', 'IVGR', 'IVGB', 'GM', 'RM', 'BM']


def build_patterns():
    t5 = np.array([1, 2, 3, 2, 1], np.float32) / 9.0

    def mfun(ch, rp, cp):
        return {
            'r': float(rp == 1 and cp == 0),
            'b': float(rp == 0 and cp == 1),
            'gr': float(rp == 1 and cp == 1),
            'gb': float(rp == 0 and cp == 0),
            'g': float((rp == 1 and cp == 1) or (rp == 0 and cp == 0)),
        }[ch]

    P, W = 128, 776
    pp = np.arange(P)[:, None] % 2
    cc = np.arange(W)[None, :] % 2  # abs col parity == m%2 (offset -4 even)
    pats = []
    for ch in ['r', 'b', 'g', 'gr', 'gb']:
        v = np.zeros((2, 2), np.float32)
        for rp in range(2):
            for cp in range(2):
                d = sum(t5[dy + 2] * t5[dx + 2]
                        * mfun(ch, (rp + dy) % 2, (cp + dx) % 2)
                        for dy in range(-2, 3) for dx in range(-2, 3))
                v[rp, cp] = 1.0 / max(d, EPS)
        pats.append(v[pp, cc].astype(np.float32))
    masks = []
    for ch in ['r', 'g', 'b']:
        v = np.array([[mfun(ch, rp, cp) for cp in range(2)]
                      for rp in range(2)], np.float32)
        masks.append(v[pp, cc].astype(np.float32))
    return np.stack(pats), np.stack(masks)  # [5,128,776], [3,128,776] (r,g,b)


def build_rowsign(h):
    sg = np.ones((4, M1), np.float32)
    for t in range(4):
        for x in range(M1):
            r = h * 384 + 96 * t - 2 + x
            if r < 0 or r >= 768:
                sg[t, x] = -1.0
    return sg.T.copy()  # [100, 4]


# staging channel layout: two contiguous output ranges (ext channels 0,1
# [gx,gy] and 15,16 [rg,bg] are DMA'd straight from their ext tiles)
STG_A = list(range(2, 15))    # gdm..gpd
STG_B = list(range(17, 30))   # gir..highband
CH = {n: i for i, n in enumerate([
    'gx', 'gy', 'gdm', 'gda', 'grad_mag', 'lap', 'lam_max', 'lam_min',
    'aniso', 'dir_var', 'orient_e', 'r_m', 'g_m', 'b_m', 'gpd', 'rg', 'bg',
    'gir', 'dgc', 'chroma_mag', 'cdv', 'cb_e', 'sh', 'sv', 'ax', 'ay',
    'phase_e', 'sx', 'sy', 'highband'])}


# ----------------------------------------------------------------------------
# kernel builder
# ----------------------------------------------------------------------------

def build_nc():
    lhs1_np, idx = build_stage1_lhs()
    a_mats, j_mats, box_mat = build_stage2_lhs()
    k3, _, _ = build_kernels()
    n1 = lhs1_np.shape[0]

    nc = bacc.Bacc(None, target_bir_lowering=False)
    bayer_d = nc.dram_tensor('bayer_pad', [392, 776], BF16, kind='ExternalInput')
    lhs1_d = nc.dram_tensor('lhs1', [K1, n1 * M1], BF16, kind='ExternalInput')
    lhs2_d = nc.dram_tensor('lhs2', [K2, 5 * 100 + 5 * 96 + 96], BF16,
                            kind='ExternalInput')
    pat_d = nc.dram_tensor('pats', [128, 5 * 776], F32, kind='ExternalInput')
    mask_d = nc.dram_tensor('masks', [128, 3 * 776], BF16,
                            kind='ExternalInput')
    rsg_d = nc.dram_tensor('rowsgn', [M1, 4], F32, kind='ExternalInput')
    out_d = nc.dram_tensor('out', [30, 384, 768], F32, kind='ExternalOutput')

    with tile.TileContext(nc) as tc, ExitStack() as ctx:
        cpool = ctx.enter_context(tc.tile_pool(name='const', bufs=1))
        inpool = ctx.enter_context(tc.tile_pool(name='inp', bufs=3))
        wpool = ctx.enter_context(tc.tile_pool(name='work', bufs=1))
        tpool = ctx.enter_context(tc.tile_pool(name='tmp', bufs=1))
        spool = ctx.enter_context(tc.tile_pool(name='stage', bufs=2))
        pspool = ctx.enter_context(
            tc.tile_pool(name='ps', bufs=8, space='PSUM'))

        epsT = cpool.tile([128, 1], F32, tag='epsT', name='epsT')
        eps4T = cpool.tile([128, 1], F32, tag='eps4T', name='eps4T')
        nc.vector.memset(epsT[:], EPS)
        nc.vector.memset(eps4T[:], 4.0 * EPS)
        lhs1_t = cpool.tile([K1, n1 * M1], BF16, tag='lhs1')
        lhs2_t = cpool.tile([K2, 1076], BF16, tag='lhs2')
        pat_t = cpool.tile([128, 5, 776], F32, tag='pats')
        mask_t = cpool.tile([128, 3, 776], BF16, tag='masks')
        rsg_t = cpool.tile([M1, 4], F32, tag='rsg')
        nc.sync.dma_start(lhs1_t[:], lhs1_d[:])
        nc.sync.dma_start(lhs2_t[:], lhs2_d[:])

        def l1(name, i=0):
            s, _ = idx[name]
            M = idx[name + '_M']
            return lhs1_t[:, (s + i) * M1:(s + i) * M1 + M]

        def l2A(i):
            return lhs2_t[:, i * 100:(i + 1) * 100]

        def l2J(i):
            return lhs2_t[:, 500 + i * 96:500 + (i + 1) * 96]

        l2box = lambda: lhs2_t[:, 980:1076]

        def MM(ps, lh, rh, start, stop):
            nc.tensor.matmul(ps, lh, rh, start=start, stop=stop)

        def stt(out, in0, w, in1):
            nc.vector.scalar_tensor_tensor(out, in0, float(w), in1,
                                           AL.mult, AL.add)

        def hconv(dst, taps, tmps):
            n = len(taps)
            if n == 1:
                nc.vector.tensor_scalar(dst, taps[0][0], float(taps[0][1]),
                                        None, AL.mult)
                return
            cur = tmps[0]
            nc.vector.tensor_scalar(cur, taps[0][0], float(taps[0][1]),
                                    None, AL.mult)
            for i in range(1, n - 1):
                nxt = tmps[i % 2]
                if nxt is cur:
                    nxt = tmps[(i + 1) % 2]
                stt(nxt, taps[i][0], taps[i][1], cur)
                cur = nxt
            stt(dst, taps[n - 1][0], taps[n - 1][1], cur)

        def act(out, in_, func, bias=0.0, scale=1.0):
            if isinstance(bias, float) and bias != 0.0:
                bt = eps4T if bias == 4.0 * EPS else epsT
                bias = bt[0:out.shape[0], :]
            nc.scalar.activation(out, in_, func, bias=bias, scale=scale)

        gx_w = [(dx, float(k3[0][1, dx + 1]) / 2.0) for dx in (-1, 1)]
        gy_w = [(dx, float(k3[1][2, dx + 1])) for dx in (-1, 0, 1)]
        t5 = np.array([1, 2, 3, 2, 1], np.float32) / 9.0

        for t in range(4):
            r0 = 96 * t
            for cbi in range(2):
                c0 = 384 * cbi
                BT = inpool.tile([K1, 392], BF16, tag='BT')
                nc.sync.dma_start(BT[:], bayer_d[r0:r0 + 104, c0:c0 + 392])
                if t == 0 and cbi == 0:
                    nc.sync.dma_start(
                        pat_t[:], pat_d[:].rearrange('p (n w) -> p n w', n=5))
                    nc.sync.dma_start(
                        mask_t[:], mask_d[:].rearrange('p (n w) -> p n w', n=3))
                    nc.sync.dma_start(rsg_t[:], rsg_d[:])

                def pv(pi, w0, wn, pn):
                    # pattern view: partitions [0:pn], master col c0+w0..c0+wn
                    return pat_t[0:pn, pi, c0 + w0:c0 + wn]

                stA = spool.tile([96, 11, 384], F32, tag='stA')
                stB = spool.tile([96, 10, 384], F32, tag='stB')
                STB_CH = [20, 21, 22, 23, 26, 27, 28, 29, 17, 18]

                def stg(name):
                    i = CH[name]
                    if i < 11:
                        return stA[:, i, :]
                    return stB[:, STB_CH.index(i), :]

                def ps_new(shape, tag='ps'):
                    return pspool.tile(shape, F32, tag=tag, name=tag)

                def tmp(tag, shape=(96, 384), dt=F32):
                    return tpool.tile(list(shape), dt, tag=tag, name=tag)

                def cser(name, pstile):
                    """central direct series: psum [96,384]"""
                    s, cnt = idx[name]
                    dxs = idx.get(name + '_dx', [0] * cnt)
                    for i in range(cnt):
                        off = 4 + dxs[i]
                        MM(pstile[:], l1(name, i), BT[:, off:off + 384],
                           i == 0, i == cnt - 1)

                def vser(name, pstile):
                    """ext-V series: psum [100,392]"""
                    MM(pstile[:], l1(name), BT[:, 0:392], True, True)

                hA = tmp('hA', (M1, 392))
                hB = tmp('hB', (M1, 392))

                # ============ fills chain
                vo_ps = ps_new([M1, 392])
                vser('Vodd', vo_ps)
                ve_ps = ps_new([M1, 392])
                vser('Veven', ve_ps)
                voddE = wpool.tile([M1, 392], BF16, tag='voddE')
                vevenE = wpool.tile([M1, 392], BF16, tag='vevenE')
                act(voddE[:], vo_ps[:], AF.Copy)
                act(vevenE[:], ve_ps[:], AF.Copy)

                AeO = ps_new([M1, 388])
                AoO = ps_new([M1, 388])
                AeE = ps_new([M1, 388])
                AoE = ps_new([M1, 388])
                for i, dx in enumerate(range(-2, 3)):
                    for src_, pse, pso in ((voddE, AeO, AoO),
                                           (vevenE, AeE, AoE)):
                        p = pse if dx % 2 == 0 else pso
                        MM(p[:], l2A(i), src_[:, 2 + dx:390 + dx],
                           dx in (-2, -1), dx in (1, 2))
                AeES = wpool.tile([M1, 388], F32, tag='AeES')
                AoES = wpool.tile([M1, 388], F32, tag='AoES')
                act(AeES[:], AeE[:], AF.Copy)
                act(AoES[:], AoE[:], AF.Copy)

                hfp = ps_new([M2, 384])
                cser('hf', hfp)
                dctp = ps_new([M2, 384])
                cser('dct', dctp)
                hf2 = tmp('hf2', dt=BF16)
                dc2 = tmp('dc2', dt=BF16)
                act(hf2[:], hfp[:], AF.Square)
                act(dc2[:], dctp[:], AF.Square)
                hbq = tmp('hbq', dt=BF16)
                nc.vector.tensor_add(hbq[:], hf2[:], dc2[:])
                act(stg('highband'), hbq[:], AF.Sqrt, bias=EPS)

                cbp = ps_new([M2, 384])
                cser('cb', cbp)
                act(stg('cb_e'), cbp[:], AF.Abs)
                shp = ps_new([M2, 384])
                cser('sh', shp)
                svp = ps_new([M2, 384])
                cser('sv', svp)
                nc.vector.tensor_copy(stg('sh'), shp[:])
                nc.vector.tensor_copy(stg('sv'), svp[:])
                sxp = ps_new([M2, 384])
                cser('sinx', sxp)
                syp = ps_new([M2, 384])
                cser('siny', syp)
                act(stg('sx'), sxp[:], AF.Copy)
                act(stg('sy'), syp[:], AF.Copy)
                pxp = ps_new([M2, 384])
                cser('phx', pxp)
                pyp = ps_new([M2, 384])
                cser('phy', pyp)
                px2 = tmp('px2', dt=BF16)
                py2 = tmp('py2', dt=BF16)
                act(px2[:], pxp[:], AF.Square)
                act(py2[:], pyp[:], AF.Square)
                phq = tmp('phq', dt=BF16)
                nc.vector.tensor_add(phq[:], px2[:], py2[:])
                act(stg('phase_e'), phq[:], AF.Sqrt, bias=EPS)

                g45p = ps_new([M2, 384])
                cser('g45', g45p)
                g135p = ps_new([M2, 384])
                cser('g135', g135p)
                o1 = tmp('o1', dt=BF16)
                o2 = tmp('o2', dt=BF16)
                act(o1[:], g45p[:], AF.Square)
                act(o2[:], g135p[:], AF.Square)
                oq = tmp('oq', dt=BF16)
                nc.vector.tensor_add(oq[:], o1[:], o2[:])
                act(stg('orient_e'), oq[:], AF.Sqrt, bias=EPS)

                ghp = ps_new([M2, 384])
                cser('gh', ghp)
                gvp = ps_new([M2, 384])
                cser('gv', gvp)
                ghS = tmp('ghS')
                act(ghS[:], ghp[:], AF.Copy)
                tdg = tmp('tdg')
                nc.vector.scalar_tensor_tensor(tdg[:], gvp[:], -1.0,
                                               ghS[:], AL.mult, AL.add)
                nc.vector.scalar_tensor_tensor(stg('dgc'), tdg[:], -1.0,
                                               tdg[:], AL.mult, AL.max)

                gxc = ps_new([M2, 384])
                cser('gxC', gxc)
                gyc = ps_new([M2, 384])
                cser('gyC', gyc)
                act(stg('gx'), gxc[:], AF.Copy)
                act(stg('gy'), gyc[:], AF.Copy)
                sqx = tmp('sqx', dt=BF16)
                sqy = tmp('sqy', dt=BF16)
                act(sqx[:], gxc[:], AF.Square)
                act(sqy[:], gyc[:], AF.Square)
                ssq = tmp('ssq', dt=BF16)
                nc.vector.tensor_add(ssq[:], sqx[:], sqy[:])
                act(stg('grad_mag'), ssq[:], AF.Sqrt, bias=EPS)

                gdm = ps_new([M2, 384])
                cser('gdm', gdm)
                gda = ps_new([M2, 384])
                cser('gda', gda)
                smd = ps_new([M2, 384])
                cser('sumd', smd)
                nc.vector.tensor_copy(stg('gdm'), gdm[:])
                nc.vector.tensor_copy(stg('gda'), gda[:])
                sq1 = tmp('sq1', dt=BF16)
                sq2 = tmp('sq2', dt=BF16)
                act(sq1[:], gdm[:], AF.Square)
                act(sq2[:], gda[:], AF.Square)
                qa = tmp('qa', dt=BF16)
                qb = tmp('qb', dt=BF16)
                nc.vector.tensor_add(qa[:], ssq[:], sq1[:])
                nc.vector.tensor_add(qb[:], qa[:], sq2[:])
                msq = tmp('msq', dt=BF16)
                act(msq[:], smd[:], AF.Square, scale=0.25)
                nc.vector.scalar_tensor_tensor(stg('dir_var'), qb[:], 0.25,
                                               msq[:], AL.mult, AL.subtract)

                lap = ps_new([M2, 384])
                cser('lap', lap)
                hdp = ps_new([M2, 384])
                cser('hd', hdp)
                hxyp = ps_new([M2, 384])
                cser('hxy', hxyp)
                nc.vector.tensor_copy(stg('lap'), lap[:])
                hd2 = tmp('hd2', dt=BF16)
                hxy2 = tmp('hxy2', dt=BF16)
                act(hd2[:], hdp[:], AF.Square, scale=0.5)
                act(hxy2[:], hxyp[:], AF.Square)
                hq = tmp('hq', dt=BF16)
                nc.vector.tensor_add(hq[:], hd2[:], hxy2[:])
                hs = tmp('hs')
                act(hs[:], hq[:], AF.Sqrt, bias=EPS)
                nc.vector.scalar_tensor_tensor(stg('lam_max'), lap[:], 0.5,
                                               hs[:], AL.mult, AL.add)
                nc.vector.scalar_tensor_tensor(stg('lam_min'), lap[:], 0.5,
                                               hs[:], AL.mult, AL.subtract)

                v3a = ps_new([M1, 392])
                vser('V3a', v3a)
                v3b = ps_new([M1, 392])
                vser('V3b', v3b)
                gx = wpool.tile([M1, 388], F32, tag='gx')
                gy = wpool.tile([M1, 388], F32, tag='gy')
                hconv(gx[:], [(v3a[:, 2 + dx:390 + dx], w) for dx, w in gx_w],
                      (hA[:, :388], hB[:, :388]))
                hconv(gy[:], [(v3b[:, 2 + dx:390 + dx], w) for dx, w in gy_w],
                      (hA[:, :388], hB[:, :388]))
                gx2 = wpool.tile([M1, 388], BF16, tag='gx2')
                gy2 = wpool.tile([M1, 388], BF16, tag='gy2')
                gxy = wpool.tile([M1, 388], F32, tag='gxy')
                gxyF = wpool.tile([M1, 388], BF16, tag='gxyF')
                act(gx2[:], gx[:], AF.Square)
                act(gy2[:], gy[:], AF.Square)
                nc.vector.tensor_mul(gxy[:], gx[:], gy[:])
                rsg = rsg_t[0:M1, t:t + 1]
                if cbi == 0:
                    nc.vector.tensor_scalar(gxyF[:, 0:2], gxy[:, 0:2], rsg,
                                            -1.0, AL.mult, AL.mult)
                    nc.vector.tensor_scalar(gxyF[:, 2:388], gxy[:, 2:388],
                                            rsg, None, AL.mult)
                else:
                    nc.vector.tensor_scalar(gxyF[:, 0:386], gxy[:, 0:386],
                                            rsg, None, AL.mult)
                    nc.vector.tensor_scalar(gxyF[:, 386:388], gxy[:, 386:388],
                                            rsg, -1.0, AL.mult, AL.mult)

                gcr = ps_new([M2, 384])
                cser('gcross', gcr)
                bayc = ps_new([M2, 384])
                cser('bayC', bayc)
                bayS = tmp('bayS')
                act(bayS[:], bayc[:], AF.Copy)
                tgi = tmp('tgi')
                nc.vector.scalar_tensor_tensor(
                    tgi[:], gcr[:], -1.0, bayS[:], AL.mult, AL.add)
                nc.vector.tensor_mul(stg('gir'), tgi[:],
                                     mask_t[0:96, 1, c0 + 4:c0 + 388])

                ev = np.s_[:, 0::2]
                od = np.s_[:, 1::2]
                fr = wpool.tile([M1, 388], F32, tag='fr')
                fb = wpool.tile([M1, 388], F32, tag='fb')
                fg = wpool.tile([M1, 388], F32, tag='fg')
                IVR, IVB, IVG = pv(0, 2, 390, 100), pv(1, 2, 390, 100), \
                    pv(2, 2, 390, 100)
                IVGR, IVGB = pv(3, 2, 390, 100), pv(4, 2, 390, 100)
                nc.vector.tensor_mul(fr[ev], AeO[ev], IVR[ev])
                nc.vector.tensor_mul(fr[od], AoO[od], IVR[od])
                nc.vector.tensor_mul(fb[ev], AoE[ev], IVB[ev])
                nc.vector.tensor_mul(fb[od], AeES[od], IVB[od])
                tg = tmp('tg', (M1, 388))
                nc.vector.tensor_add(tg[ev], AoO[ev], AeES[ev])
                nc.vector.tensor_add(tg[od], AeO[od], AoES[od])
                nc.vector.tensor_mul(fg[ev], tg[ev], IVG[ev])
                nc.vector.tensor_mul(fg[od], tg[od], IVG[od])
                fgr = tmp('fgr', (M1, 388))
                fgb = tmp('fgb', (M1, 388))
                nc.vector.tensor_mul(fgr[ev], AoO[ev], IVGR[ev])
                nc.vector.tensor_mul(fgr[od], AeO[od], IVGR[od])
                nc.vector.tensor_mul(fgb[ev], AeES[ev], IVGB[ev])
                nc.vector.tensor_mul(fgb[od], AoES[od], IVGB[od])
                gpdE = wpool.tile([M1, 388], F32, tag='gpdE')
                nc.vector.tensor_sub(gpdE[:], fgr[:], fgb[:])

                rg = wpool.tile([M1, 388], F32, tag='rg')
                bg = wpool.tile([M1, 388], F32, tag='bg')
                nc.vector.tensor_sub(rg[:], fr[:], fg[:])
                nc.vector.tensor_sub(bg[:], fb[:], fg[:])
                rg2 = wpool.tile([M1, 388], BF16, tag='rg2')
                bg2 = wpool.tile([M1, 388], BF16, tag='bg2')
                act(rg2[:], rg[:], AF.Square)
                act(bg2[:], bg[:], AF.Square)
                rgB = wpool.tile([M1, 388], BF16, tag='rgB')
                bgB = wpool.tile([M1, 388], BF16, tag='bgB')
                nc.vector.tensor_copy(rgB[:], rg[:])
                nc.vector.tensor_copy(bgB[:], bg[:])
                cq = tmp('cq', (M1, 388), dt=BF16)
                nc.vector.tensor_add(cq[:], rg2[:], bg2[:])
                chromE = wpool.tile([M1, 388], F32, tag='chromE')
                act(chromE[:], cq[:], AF.Sqrt, bias=EPS)

                # box + cdv
                bx = {}
                for nm, src_ in (('m1r', rgB), ('m1b', bgB),
                                 ('m2r', rg2), ('m2b', bg2)):
                    p = ps_new([M2, 384])
                    for i, dx in enumerate(range(-2, 3)):
                        MM(p[:], l2box(), src_[:, 2 + dx:386 + dx],
                           i == 0, i == 4)
                    bx[nm] = p
                q1 = tmp('q1')
                q2 = tmp('q2')
                act(q1[:], bx['m1r'][:], AF.Square)
                act(q2[:], bx['m1b'][:], AF.Square)
                v1 = tmp('v1')
                v2 = tmp('v2')
                stt(v1[:], q1[:], -1.0, bx['m2r'][:])
                stt(v2[:], q2[:], -1.0, bx['m2b'][:])
                v1m = tmp('v1m')
                v2m = tmp('v2m')
                nc.vector.tensor_scalar(v1m[:], v1[:], 0.0, None, AL.max)
                nc.vector.tensor_scalar(v2m[:], v2[:], 0.0, None, AL.max)
                nc.vector.tensor_add(stg('cdv'), v1m[:], v2m[:])

                # ============ ext gradients -> J -> aniso
                jps = {nm: ps_new([M2, 384]) for nm in
                       ('Jxx', 'Jyy', 'Jxy')}
                jsrc = {'Jxx': gx2, 'Jyy': gy2, 'Jxy': gxyF}
                for i, dx in enumerate(range(-2, 3)):
                    for nm in ('Jxx', 'Jyy', 'Jxy'):
                        MM(jps[nm][:], l2J(i),
                           jsrc[nm][:, 2 + dx:386 + dx], i == 0, i == 4)
                jyyS = tmp('jyyS')
                act(jyyS[:], jps['Jyy'][:], AF.Copy)
                dj = tmp('dj')
                sm = tmp('sm')
                nc.vector.scalar_tensor_tensor(dj[:], jps['Jxx'][:], 1.0,
                                               jyyS[:], AL.mult, AL.subtract)
                nc.vector.scalar_tensor_tensor(sm[:], jps['Jxx'][:], 1.0,
                                               jyyS[:], AL.mult, AL.add)
                dj2 = tmp('dj2', dt=BF16)
                jxy2 = tmp('jxy2', dt=BF16)
                act(dj2[:], dj[:], AF.Square, scale=0.5)
                act(jxy2[:], jps['Jxy'][:], AF.Square)
                qj = tmp('qj', dt=BF16)
                nc.vector.tensor_add(qj[:], dj2[:], jxy2[:])
                anum = tmp('anum')
                act(anum[:], qj[:], AF.Sqrt, bias=4.0 * EPS, scale=4.0)
                sme = tmp('sme')
                nc.vector.tensor_scalar(sme[:], sm[:], EPS, None, AL.add)
                rec = tmp('rec')
                nc.vector.reciprocal(rec[:], sme[:])
                nc.vector.tensor_mul(stg('aniso'), anum[:], rec[:])

                # ============ central channels
                nc.gpsimd.dma_start(
                    out_d[11:14, r0:r0 + 96, c0:c0 + 384]
                    .rearrange('n p w -> p n w'),
                    mask_t[0:96, 0:3, c0 + 4:c0 + 388])

                # ============ output DMAs
                nc.gpsimd.dma_start(
                    out_d[0:11, r0:r0 + 96, c0:c0 + 384]
                    .rearrange('n p w -> p n w'), stA[:])
                # stB: [cdv, cb_e, sh, sv, phase, sx, sy, hb, gir, dgc]
                nc.gpsimd.dma_start(
                    out_d[20:24, r0:r0 + 96, c0:c0 + 384]
                    .rearrange('n p w -> p n w'), stB[:, 0:4, :])
                nc.gpsimd.dma_start(
                    out_d[24:26, r0:r0 + 96, c0:c0 + 384]
                    .rearrange('n p w -> p n w'), stB[:, 2:4, :])
                nc.gpsimd.dma_start(
                    out_d[26:30, r0:r0 + 96, c0:c0 + 384]
                    .rearrange('n p w -> p n w'), stB[:, 4:8, :])
                nc.gpsimd.dma_start(
                    out_d[17:19, r0:r0 + 96, c0:c0 + 384]
                    .rearrange('n p w -> p n w'), stB[:, 8:10, :])
                CENV = np.s_[2:98, 2:386]
                nc.gpsimd.dma_start(out_d[14, r0:r0 + 96, c0:c0 + 384],
                                  gpdE[CENV])
                nc.gpsimd.dma_start(out_d[15, r0:r0 + 96, c0:c0 + 384],
                                  rg[CENV])
                nc.gpsimd.dma_start(out_d[16, r0:r0 + 96, c0:c0 + 384],
                                  bg[CENV])
                nc.gpsimd.dma_start(out_d[19, r0:r0 + 96, c0:c0 + 384],
                                  chromE[CENV])

    nc.compile()
    return nc, lhs1_np, (a_mats, j_mats, box_mat), n1


_STATE = {}


def _get_state():
    if 'nc' not in _STATE:
        nc, lhs1_np, (a_mats, j_mats, box_mat), n1 = build_nc()
        lhs1_pack = np.ascontiguousarray(
            lhs1_np.transpose(1, 0, 2).reshape(K1, n1 * M1))
        lhs2_pack = np.concatenate(
            [a_mats.transpose(1, 0, 2).reshape(K2, 500),
             j_mats.transpose(1, 0, 2).reshape(K2, 480),
             box_mat], axis=1).astype(np.float32)
        import ml_dtypes
        pats, masks = build_patterns()
        pat_pack = np.ascontiguousarray(
            pats.transpose(1, 0, 2).reshape(128, 5 * 776))
        mask_pack = np.ascontiguousarray(
            masks.transpose(1, 0, 2).reshape(128, 3 * 776)
            .astype(ml_dtypes.bfloat16))
        import ml_dtypes
        _STATE.update(nc=nc,
                      lhs1=np.ascontiguousarray(
                          lhs1_pack.astype(ml_dtypes.bfloat16)),
                      lhs2=np.ascontiguousarray(
                          lhs2_pack.astype(ml_dtypes.bfloat16)),
                      pats=pat_pack, masks=mask_pack,
                      rsg=[np.ascontiguousarray(build_rowsign(0)),
                           np.ascontiguousarray(build_rowsign(1))])
    return _STATE


def _run(bayer, trace=False, **kw):
    st = _get_state()
    bayer = np.ascontiguousarray(np.asarray(bayer, dtype=np.float32))
    in_maps = []
    for core in range(8):
        b, h = core // 2, core % 2
        Pimg = np.pad(bayer[b, 0], 4, mode='reflect')
        import ml_dtypes
        bp = np.ascontiguousarray(
            Pimg[h * 384:h * 384 + 392, :].astype(ml_dtypes.bfloat16))
        in_maps.append({'bayer_pad': bp, 'lhs1': st['lhs1'],
                        'lhs2': st['lhs2'], 'pats': st['pats'],
                        'masks': st['masks'], 'rowsgn': st['rsg'][h]})
    res = run_bass_kernel_spmd(st['nc'], in_maps, core_ids=list(range(8)),
                               trace=trace, **kw)
    out = np.empty((4, 30, 768, 768), np.float32)
    for core in range(8):
        b, h = core // 2, core % 2
        out[b, :, h * 384:(h + 1) * 384, :] = res.results[core]['out']
    return out, res


def kernel(bayer):
    out, _ = _run(bayer, trace=False)
    return out

